# revision 5
# baseline (speedup 1.0000x reference)
"""Trainium2 Bass kernel for nn_LiquidNeuralNetwork (B=512, S=1024, IN=16, HID=64).

Strategy
--------
The reference integrates dh/dt = (-h + tanh(h) @ W_hh.T + inp + bias) / tau
with RK4 x 4 substeps per timestep (16 sequential tanh+matmul rounds per
step).  At dt = 1/1023 the integration error of far cheaper schemes is orders
of magnitude below f32 rounding noise, so we integrate the same ODE with an
exponential integrator + AB2 extrapolation of the (tiny) tanh coupling term:

    H_s = a*H_{s-1} + b*(c_s + 1.5*g_{s-1} - 0.5*g_{s-2}),
    g_s = W_hh @ tanh(H_s),  a = exp(-dt/tau), b = 1 - a,
    c_s = W_ih @ (W_in x_s + b_in) + bias   (precomputed, hidden-major)

which agrees with the reference to ~6e-6 (the f32 noise floor of the
reference itself) while needing ONE tanh + matmul round per timestep.

On-device layout: hidden on partitions, batch on free dim; batch sharded
8 ways (64 per core).  Per round the PSUM bank accumulates the full affine
update via matmuls only:

    bank_r[0:64]  = diag(b) @ c_r + diag(a) @ hm_r + [Wp;Wm] @ [th_r;th_{r-1}]
    bank_r[64]    = W_out @ th_r          (the per-step scalar output)

with Wp = (1.5*b*W_hh)^T, Wm = (-0.5*b*W_hh)^T.  tanh runs on ACT straight
from PSUM; DVE copies bank->SBUF (h materialization + output-row collection);
everything except ACT->PE->ACT is off the critical path.
"""

import os
import numpy as np

import concourse.bacc as bacc
import concourse.tile as tile
from concourse import mybir
from concourse.bass_utils import run_bass_kernel_spmd

F32 = mybir.dt.float32
H = 64          # hidden
BIN = 16        # input features
B_FULL = 512
S = int(os.environ.get("LNN_S", "1024"))   # harness always runs 1024
N_CORES = 8
B = B_FULL // N_CORES   # 64 per-core batch
SEG = 128 if S % 128 == 0 else S           # output segment length (steps)
N_SEG = S // SEG

TRACE = bool(int(os.environ.get("LNN_TRACE", "0")))
SCHEME = os.environ.get("LNN_SCHEME", "pair")   # "pair" | "e2"

NPAIR = S // 2                 # pair rounds
SEGP = NPAIR if NPAIR <= 256 else 256   # pair-slots per output segment
NSEGP = NPAIR // SEGP

_cached = {}


def _build_program():
    """Build + compile the Bass program (same NEFF for all cores)."""
    nc = bacc.Bacc("TRN2", target_bir_lowering=False, debug=False)

    in_C = nc.dram_tensor("in_C", (S, H, B), F32, kind="ExternalInput").ap()
    in_Aev = nc.dram_tensor("in_Aev", (2 * H, H + 1), F32, kind="ExternalInput").ap()
    in_Aod = nc.dram_tensor("in_Aod", (2 * H, H + 1), F32, kind="ExternalInput").ap()
    in_Atl = nc.dram_tensor("in_Atl", (2 * H, H + 1), F32, kind="ExternalInput").ap()
    in_Db = nc.dram_tensor("in_Db", (H, H + 1), F32, kind="ExternalInput").ap()
    in_Da = nc.dram_tensor("in_Da", (H, H), F32, kind="ExternalInput").ap()
    out_dram = nc.dram_tensor("out", (N_SEG, SEG * B), F32, kind="ExternalOutput").ap()

    TANH = mybir.ActivationFunctionType.Tanh

    with tile.TileContext(nc) as tc:
        with (
            tc.tile_pool(name="wts", bufs=1) as wts,
            tc.tile_pool(name="thp", bufs=1) as thp,
            tc.tile_pool(name="osb", bufs=2) as osbp,
            tc.tile_pool(name="cp", bufs=10) as cp,
            tc.tile_pool(name="hmp", bufs=3) as hmp,
            tc.tile_pool(name="hbank", bufs=4, space="PSUM") as hbank,
        ):
            t_Aev = wts.tile([2 * H, H + 1], F32, tag="aev")
            t_Aod = wts.tile([2 * H, H + 1], F32, tag="aod")
            t_Atl = wts.tile([2 * H, H + 1], F32, tag="atl")
            t_Db = wts.tile([H, H + 1], F32, tag="db")
            t_Da = wts.tile([H, H], F32, tag="da")
            nc.sync.dma_start(out=t_Aev, in_=in_Aev)
            nc.sync.dma_start(out=t_Aod, in_=in_Aod)
            nc.sync.dma_start(out=t_Atl, in_=in_Atl)
            nc.sync.dma_start(out=t_Db, in_=in_Db)
            nc.sync.dma_start(out=t_Da, in_=in_Da)

            # persistent tanh tile: half0 = th of even rounds, half1 = odd
            t_th = thp.tile([2 * H, B], F32, tag="th")
            nc.vector.memset(t_th, 0.0)

            # output staging: only partition 64 is used; slot o at free
            # offset (o % SEG)*B.  Two tiles ping-pong across segments.
            t_osb = [osbp.tile([H + 1, SEG * B], F32, tag="osb", name=f"t_osb{i}")
                     for i in range(2)]

            prev_bank = None
            for r in range(1, S):
                t_c = cp.tile([H, B], F32, tag="c")
                nc.sync.dma_start(out=t_c, in_=in_C[r])

                bank = hbank.tile([H + 1, B], F32, tag="bank")
                last = r == 1
                # M4 first (start=True): clears rows 0..64 (col H of Db is 0)
                nc.tensor.matmul(bank, t_Db, t_c, start=True, stop=last)

                if r >= 2:
                    o = r - 2          # output index evacuated this round
                    seg, slot = divmod(o, SEG)
                    # evacuate prev bank's output row (lane-aligned copy)
                    nc.vector.tensor_copy(
                        t_osb[seg % 2][H:H + 1, slot * B:(slot + 1) * B],
                        prev_bank[H:H + 1, :],
                    )
                    if slot == SEG - 1:
                        nc.sync.dma_start(
                            out=out_dram[seg],
                            in_=t_osb[seg % 2][H:H + 1, :],
                        )
                    # h materialization for the decay term
                    t_hm = hmp.tile([H, B], F32, tag="hm")
                    nc.vector.tensor_copy(t_hm, prev_bank[:H, :])
                    # tanh straight from PSUM into this round's th half
                    half = r % 2
                    nc.scalar.activation(
                        t_th[half * H:(half + 1) * H, :], prev_bank[:H, :], TANH)
                    nc.tensor.matmul(bank[:H, :], t_Da, t_hm,
                                     start=False, stop=False)
                    t_A = t_Aev if r % 2 == 0 else t_Aod
                    nc.tensor.matmul(bank, t_A, t_th, start=False, stop=True)
                prev_bank = bank

            # tail: evacuate out_{S-2}; th_S = tanh(H_{S-1}); out_{S-1}
            o = S - 2
            seg, slot = divmod(o, SEG)
            nc.vector.tensor_copy(
                t_osb[seg % 2][H:H + 1, slot * B:(slot + 1) * B],
                prev_bank[H:H + 1, :],
            )
            half = S % 2
            nc.scalar.activation(
                t_th[half * H:(half + 1) * H, :], prev_bank[:H, :], TANH)
            tbank = hbank.tile([H + 1, B], F32, tag="bank")
            nc.tensor.matmul(tbank, t_Atl, t_th, start=True, stop=True)
            o = S - 1
            seg, slot = divmod(o, SEG)
            nc.vector.tensor_copy(
                t_osb[seg % 2][H:H + 1, slot * B:(slot + 1) * B],
                tbank[H:H + 1, :],
            )
            nc.sync.dma_start(out=out_dram[seg], in_=t_osb[seg % 2][H:H + 1, :])

    nc.compile()
    return nc


def _host_precompute(x, W_in, b_in, W_hh, W_ih, bias, tau, W_out, b_out):
    x = np.asarray(x, dtype=np.float32)
    W_in = np.asarray(W_in, dtype=np.float32)
    b_in = np.asarray(b_in, dtype=np.float32)
    W_hh = np.asarray(W_hh, dtype=np.float32)
    W_ih = np.asarray(W_ih, dtype=np.float32)
    bias = np.asarray(bias, dtype=np.float32)
    tau = np.asarray(tau, dtype=np.float32)
    W_out = np.asarray(W_out, dtype=np.float32)

    W_comb = W_ih @ W_in                      # [H, BIN]
    b_comb = W_ih @ b_in + bias               # [H]
    C = x @ W_comb.T + b_comb                 # [B_FULL, S, H] f32

    t = np.linspace(0.0, 1.0, S).astype(np.float32)
    dt = np.float64(t[1]) - np.float64(t[0])
    d = 1.0 / tau.astype(np.float64)
    a = np.exp(-d * dt)
    b = 1.0 - a

    Wp = (1.5 * b[:, None] * W_hh.astype(np.float64)).T   # lhsT [k, j]
    Wm = (-0.5 * b[:, None] * W_hh.astype(np.float64)).T
    wout = W_out[0].astype(np.float64)                    # [H]

    Aev = np.zeros((2 * H, H + 1), np.float64)
    Aev[:H, :H] = Wp
    Aev[H:, :H] = Wm
    Aev[:H, H] = wout
    Aod = np.zeros((2 * H, H + 1), np.float64)
    Aod[:H, :H] = Wm
    Aod[H:, :H] = Wp
    Aod[H:, H] = wout
    # tail round index S (=1024, even): th_S lives in half S%2
    Atl = np.zeros((2 * H, H + 1), np.float64)
    if S % 2 == 0:
        Atl[:H, H] = wout
    else:
        Atl[H:, H] = wout
    Db = np.zeros((H, H + 1), np.float64)
    Db[:, :H] = np.diag(b)
    Da = np.diag(a)

    return C, {
        "in_Aev": Aev.astype(np.float32),
        "in_Aod": Aod.astype(np.float32),
        "in_Atl": Atl.astype(np.float32),
        "in_Db": Db.astype(np.float32),
        "in_Da": Da.astype(np.float32),
    }


def kernel(x, W_in, b_in, W_hh, W_ih, bias, tau, W_out, b_out):
    C, wmaps = _host_precompute(x, W_in, b_in, W_hh, W_ih, bias, tau,
                                W_out, b_out)
    b_out = np.asarray(b_out, dtype=np.float32)

    if "nc" not in _cached:
        _cached["nc"] = _build_program()
    nc = _cached["nc"]

    in_maps = []
    for i in range(N_CORES):
        C_core = np.ascontiguousarray(
            C[i * B:(i + 1) * B].transpose(1, 2, 0))     # [S, H, B]
        in_maps.append({"in_C": C_core, **wmaps})

    core_ids = list(range(N_CORES))
    res = run_bass_kernel_spmd(nc, in_maps, core_ids)

    out = np.empty((B_FULL, S, 1), dtype=np.float32)
    for i in range(N_CORES):
        dev = res.results[i]["out"].reshape(S, B)        # [s, b_local]
        out[i * B:(i + 1) * B, :, 0] = dev.T + b_out[0]
    return out


# revision 6
# speedup vs baseline: 10.8964x; 10.8964x over previous
"""Trainium2 Bass kernel for nn_LiquidNeuralNetwork (B=512, S=1024, IN=16, HID=64).

Strategy
--------
The reference integrates dh/dt = (-h + tanh(h) @ W_hh.T + inp + bias) / tau
with RK4 x 4 substeps per timestep (16 sequential tanh+matmul rounds per
step).  At dt = 1/1023 the integration error of far cheaper schemes is orders
of magnitude below f32 rounding noise, so we integrate the same ODE with an
exponential integrator + AB2 extrapolation of the (tiny) tanh coupling term:

    H_s = a*H_{s-1} + b*(c_s + 1.5*g_{s-1} - 0.5*g_{s-2}),
    g_s = W_hh @ tanh(H_s),  a = exp(-dt/tau), b = 1 - a,
    c_s = W_ih @ (W_in x_s + b_in) + bias   (precomputed, hidden-major)

which agrees with the reference to ~6e-6 (the f32 noise floor of the
reference itself) while needing ONE tanh + matmul round per timestep.

On-device layout: hidden on partitions, batch on free dim; batch sharded
8 ways (64 per core).  Per round the PSUM bank accumulates the full affine
update via matmuls only:

    bank_r[0:64]  = diag(b) @ c_r + diag(a) @ hm_r + [Wp;Wm] @ [th_r;th_{r-1}]
    bank_r[64]    = W_out @ th_r          (the per-step scalar output)

with Wp = (1.5*b*W_hh)^T, Wm = (-0.5*b*W_hh)^T.  tanh runs on ACT straight
from PSUM; DVE copies bank->SBUF (h materialization + output-row collection);
everything except ACT->PE->ACT is off the critical path.
"""

import os
import numpy as np

import concourse.bacc as bacc
import concourse.tile as tile
from concourse import mybir
from concourse.bass_utils import run_bass_kernel_spmd

F32 = mybir.dt.float32
H = 64          # hidden
BIN = 16        # input features
B_FULL = 512
S = int(os.environ.get("LNN_S", "1024"))   # harness always runs 1024
N_CORES = 8
B = B_FULL // N_CORES   # 64 per-core batch
SEG = 128 if S % 128 == 0 else S           # output segment length (steps)
N_SEG = S // SEG

TRACE = bool(int(os.environ.get("LNN_TRACE", "0")))
SCHEME = os.environ.get("LNN_SCHEME", "pair")   # "pair" | "e2"

NPAIR = S // 2                 # pair rounds
SEGP = NPAIR if NPAIR <= 256 else 256   # pair-slots per output segment
NSEGP = NPAIR // SEGP

_cached = {}


def _build_program():
    """Build + compile the Bass program (same NEFF for all cores)."""
    nc = bacc.Bacc("TRN2", target_bir_lowering=False, debug=False)

    in_C = nc.dram_tensor("in_C", (S, H, B), F32, kind="ExternalInput").ap()
    in_Aev = nc.dram_tensor("in_Aev", (2 * H, H + 1), F32, kind="ExternalInput").ap()
    in_Aod = nc.dram_tensor("in_Aod", (2 * H, H + 1), F32, kind="ExternalInput").ap()
    in_Atl = nc.dram_tensor("in_Atl", (2 * H, H + 1), F32, kind="ExternalInput").ap()
    in_Db = nc.dram_tensor("in_Db", (H, H + 1), F32, kind="ExternalInput").ap()
    in_Da = nc.dram_tensor("in_Da", (H, H), F32, kind="ExternalInput").ap()
    out_dram = nc.dram_tensor("out", (N_SEG, SEG * B), F32, kind="ExternalOutput").ap()

    TANH = mybir.ActivationFunctionType.Tanh

    with tile.TileContext(nc) as tc:
        with (
            tc.tile_pool(name="wts", bufs=1) as wts,
            tc.tile_pool(name="thp", bufs=1) as thp,
            tc.tile_pool(name="osb", bufs=2) as osbp,
            tc.tile_pool(name="cp", bufs=10) as cp,
            tc.tile_pool(name="hmp", bufs=3) as hmp,
            tc.tile_pool(name="hbank", bufs=4, space="PSUM") as hbank,
        ):
            t_Aev = wts.tile([2 * H, H + 1], F32, tag="aev")
            t_Aod = wts.tile([2 * H, H + 1], F32, tag="aod")
            t_Atl = wts.tile([2 * H, H + 1], F32, tag="atl")
            t_Db = wts.tile([H, H + 1], F32, tag="db")
            t_Da = wts.tile([H, H], F32, tag="da")
            nc.sync.dma_start(out=t_Aev, in_=in_Aev)
            nc.sync.dma_start(out=t_Aod, in_=in_Aod)
            nc.sync.dma_start(out=t_Atl, in_=in_Atl)
            nc.sync.dma_start(out=t_Db, in_=in_Db)
            nc.sync.dma_start(out=t_Da, in_=in_Da)

            # persistent tanh tile: half0 = th of even rounds, half1 = odd
            t_th = thp.tile([2 * H, B], F32, tag="th")
            nc.vector.memset(t_th, 0.0)

            # output staging: only partition 64 is used; slot o at free
            # offset (o % SEG)*B.  Two tiles ping-pong across segments.
            t_osb = [osbp.tile([H + 1, SEG * B], F32, tag="osb", name=f"t_osb{i}")
                     for i in range(2)]

            prev_bank = None
            for r in range(1, S):
                t_c = cp.tile([H, B], F32, tag="c")
                nc.sync.dma_start(out=t_c, in_=in_C[r])

                bank = hbank.tile([H + 1, B], F32, tag="bank")
                last = r == 1
                # M4 first (start=True): clears rows 0..64 (col H of Db is 0)
                nc.tensor.matmul(bank, t_Db, t_c, start=True, stop=last)

                if r >= 2:
                    o = r - 2          # output index evacuated this round
                    seg, slot = divmod(o, SEG)
                    # evacuate prev bank's output row (lane-aligned copy)
                    nc.vector.tensor_copy(
                        t_osb[seg % 2][H:H + 1, slot * B:(slot + 1) * B],
                        prev_bank[H:H + 1, :],
                    )
                    if slot == SEG - 1:
                        nc.sync.dma_start(
                            out=out_dram[seg],
                            in_=t_osb[seg % 2][H:H + 1, :],
                        )
                    # h materialization for the decay term
                    t_hm = hmp.tile([H, B], F32, tag="hm")
                    nc.vector.tensor_copy(t_hm, prev_bank[:H, :])
                    # tanh straight from PSUM into this round's th half
                    half = r % 2
                    nc.scalar.activation(
                        t_th[half * H:(half + 1) * H, :], prev_bank[:H, :], TANH)
                    nc.tensor.matmul(bank[:H, :], t_Da, t_hm,
                                     start=False, stop=False)
                    t_A = t_Aev if r % 2 == 0 else t_Aod
                    nc.tensor.matmul(bank, t_A, t_th, start=False, stop=True)
                prev_bank = bank

            # tail: evacuate out_{S-2}; th_S = tanh(H_{S-1}); out_{S-1}
            o = S - 2
            seg, slot = divmod(o, SEG)
            nc.vector.tensor_copy(
                t_osb[seg % 2][H:H + 1, slot * B:(slot + 1) * B],
                prev_bank[H:H + 1, :],
            )
            half = S % 2
            nc.scalar.activation(
                t_th[half * H:(half + 1) * H, :], prev_bank[:H, :], TANH)
            tbank = hbank.tile([H + 1, B], F32, tag="bank")
            nc.tensor.matmul(tbank, t_Atl, t_th, start=True, stop=True)
            o = S - 1
            seg, slot = divmod(o, SEG)
            nc.vector.tensor_copy(
                t_osb[seg % 2][H:H + 1, slot * B:(slot + 1) * B],
                tbank[H:H + 1, :],
            )
            nc.sync.dma_start(out=out_dram[seg], in_=t_osb[seg % 2][H:H + 1, :])

    nc.compile()
    return nc


def _build_program_pair():
    """Pair-corrected scheme: 2 timesteps per tanh round (S/2 rounds).

    PSUM bank halves = [H_s ; H_{s+1}^pred]; one ACT tanh covers both.
    State propagation is exact AB2 (prediction error of odd tanh inputs is
    corrected one round later via deeper tanh history), validated ~9e-6.
    """
    nc = bacc.Bacc("TRN2", target_bir_lowering=False, debug=False)

    in_C = nc.dram_tensor("in_C", (S, H, B), F32, kind="ExternalInput").ap()
    ins = {}
    for nm in ("LC", "LH", "LT1", "LT2", "LB"):
        ins[nm] = nc.dram_tensor(f"in_{nm}", (2 * H, 2 * H), F32,
                                 kind="ExternalInput").ap()
    ins["LO"] = nc.dram_tensor("in_LO", (2 * H, 2), F32,
                               kind="ExternalInput").ap()
    out_dram = nc.dram_tensor("out", (NSEGP, 2, SEGP * B), F32,
                              kind="ExternalOutput").ap()

    TANH = mybir.ActivationFunctionType.Tanh

    with tile.TileContext(nc) as tc:
        with (
            tc.tile_pool(name="wts", bufs=1) as wts,
            tc.tile_pool(name="thp", bufs=4) as thp,
            tc.tile_pool(name="thz", bufs=1) as thz,
            tc.tile_pool(name="osb", bufs=2) as osbp,
            tc.tile_pool(name="cpr", bufs=8) as cpr,
            tc.tile_pool(name="hmp", bufs=3) as hmp,
            tc.tile_pool(name="hbank", bufs=4, space="PSUM") as hbank,
            tc.tile_pool(name="obank", bufs=3, space="PSUM") as obankp,
        ):
            t_w = {}
            for nm in ("LC", "LH", "LT1", "LT2", "LB"):
                t_w[nm] = wts.tile([2 * H, 2 * H], F32, name=f"t_{nm}")
                nc.sync.dma_start(out=t_w[nm], in_=ins[nm])
            t_w["LO"] = wts.tile([2 * H, 2], F32, name="t_LO")
            nc.sync.dma_start(out=t_w["LO"], in_=ins["LO"])

            t_zero = thz.tile([2 * H, B], F32, tag="t1zero")
            nc.vector.memset(t_zero, 0.0)
            t_osb = [osbp.tile([2, SEGP * B], F32, tag="osb", name=f"t_osb{i}")
                     for i in range(2)]

            # boot: bank_0 = [0 ; b*c_1]
            t_cp = cpr.tile([2 * H, B], F32, tag="cp")
            nc.sync.dma_start(out=t_cp, in_=in_C[0:2].flatten_outer_dims())
            bank = hbank.tile([2 * H, B], F32, tag="bank")
            nc.tensor.matmul(bank, t_w["LB"], t_cp, start=True, stop=True)

            prev_bank = bank
            prev_T1 = t_zero
            prev_ob = None
            for r in range(1, NPAIR):
                t_cp = cpr.tile([2 * H, B], F32, tag="cp")
                nc.sync.dma_start(out=t_cp,
                                  in_=in_C[2 * r:2 * r + 2].flatten_outer_dims())
                bank = hbank.tile([2 * H, B], F32, tag="bank")
                nc.tensor.matmul(bank, t_w["LC"], t_cp, start=True, stop=False)

                # from prev bank: tanh pair + h materialization
                T1 = thp.tile([2 * H, B], F32, tag="t1")
                nc.scalar.activation(T1, prev_bank, TANH)
                t_hm = hmp.tile([2 * H, B], F32, tag="hm")
                nc.vector.tensor_copy(t_hm, prev_bank)

                if prev_ob is not None:
                    m = r - 2
                    seg, slot = divmod(m, SEGP)
                    nc.vector.tensor_copy(
                        t_osb[seg % 2][0:2, slot * B:(slot + 1) * B], prev_ob)
                    if slot == SEGP - 1:
                        nc.sync.dma_start(out=out_dram[seg],
                                          in_=t_osb[seg % 2][0:2, :])

                nc.tensor.matmul(bank, t_w["LH"], t_hm, start=False, stop=False)
                nc.tensor.matmul(bank, t_w["LT2"], prev_T1,
                                 start=False, stop=False)
                nc.tensor.matmul(bank, t_w["LT1"], T1, start=False, stop=True)
                ob = obankp.tile([2, B], F32, tag="ob")
                nc.tensor.matmul(ob, t_w["LO"], T1, start=True, stop=True)

                prev_bank, prev_T1, prev_ob = bank, T1, ob

            # tail: last tanh pair -> outs (S-2, S-1); flush last segment
            T1 = thp.tile([2 * H, B], F32, tag="t1")
            nc.scalar.activation(T1, prev_bank, TANH)
            m = NPAIR - 2
            seg, slot = divmod(m, SEGP)
            nc.vector.tensor_copy(
                t_osb[seg % 2][0:2, slot * B:(slot + 1) * B], prev_ob)
            ob = obankp.tile([2, B], F32, tag="ob")
            nc.tensor.matmul(ob, t_w["LO"], T1, start=True, stop=True)
            m = NPAIR - 1
            seg, slot = divmod(m, SEGP)
            nc.vector.tensor_copy(
                t_osb[seg % 2][0:2, slot * B:(slot + 1) * B], ob)
            nc.sync.dma_start(out=out_dram[seg], in_=t_osb[seg % 2][0:2, :])

    nc.compile()
    return nc


def _pair_weights(a, b, W_hh, W_out):
    """Host lhsT matrices for the pair-corrected scheme (f64 in, f32 out)."""
    W = W_hh.astype(np.float64)
    wout = W_out[0].astype(np.float64)
    ab, a2, a2b = a * b, a * a, a * a * b

    def blk(v):
        return (v[:, None] * W).T

    LC = np.zeros((2 * H, 2 * H))
    LC[:H, :H] = np.diag(b)
    LC[:H, H:] = np.diag(ab)
    LC[H:, H:] = np.diag(b)
    LH = np.zeros((2 * H, 2 * H))
    LH[H:, :H] = np.diag(a)
    LH[H:, H:] = np.diag(a2)
    LT1 = np.zeros((2 * H, 2 * H))
    LT1[:H, :H] = blk(-0.5 * b + 1.5 * ab)
    LT1[:H, H:] = blk(-0.5 * ab + 1.5 * a2b - 1.5 * b)
    LT1[H:, :H] = blk(1.5 * b)
    LT1[H:, H:] = blk(1.5 * ab + 2.5 * b)
    LT2 = np.zeros((2 * H, 2 * H))
    LT2[:H, :H] = blk(1.5 * ab)
    LT2[:H, H:] = blk(1.5 * a2b)
    LT2[H:, :H] = blk(-3.0 * ab)
    LT2[H:, H:] = blk(-3.0 * a2b)
    LB = np.zeros((2 * H, 2 * H))
    LB[H:, H:] = np.diag(b)
    LO = np.zeros((2 * H, 2))
    LO[:H, 0] = wout
    LO[H:, 1] = wout
    return {f"in_{nm}": m.astype(np.float32)
            for nm, m in [("LC", LC), ("LH", LH), ("LT1", LT1),
                          ("LT2", LT2), ("LB", LB), ("LO", LO)]}


def _host_precompute(x, W_in, b_in, W_hh, W_ih, bias, tau, W_out, b_out):
    x = np.asarray(x, dtype=np.float32)
    W_in = np.asarray(W_in, dtype=np.float32)
    b_in = np.asarray(b_in, dtype=np.float32)
    W_hh = np.asarray(W_hh, dtype=np.float32)
    W_ih = np.asarray(W_ih, dtype=np.float32)
    bias = np.asarray(bias, dtype=np.float32)
    tau = np.asarray(tau, dtype=np.float32)
    W_out = np.asarray(W_out, dtype=np.float32)

    W_comb = W_ih @ W_in                      # [H, BIN]
    b_comb = W_ih @ b_in + bias               # [H]
    C = x @ W_comb.T + b_comb                 # [B_FULL, S, H] f32

    t = np.linspace(0.0, 1.0, S).astype(np.float32)
    dt = np.float64(t[1]) - np.float64(t[0])
    d = 1.0 / tau.astype(np.float64)
    a = np.exp(-d * dt)
    b = 1.0 - a

    Wp = (1.5 * b[:, None] * W_hh.astype(np.float64)).T   # lhsT [k, j]
    Wm = (-0.5 * b[:, None] * W_hh.astype(np.float64)).T
    wout = W_out[0].astype(np.float64)                    # [H]

    Aev = np.zeros((2 * H, H + 1), np.float64)
    Aev[:H, :H] = Wp
    Aev[H:, :H] = Wm
    Aev[:H, H] = wout
    Aod = np.zeros((2 * H, H + 1), np.float64)
    Aod[:H, :H] = Wm
    Aod[H:, :H] = Wp
    Aod[H:, H] = wout
    # tail round index S (=1024, even): th_S lives in half S%2
    Atl = np.zeros((2 * H, H + 1), np.float64)
    if S % 2 == 0:
        Atl[:H, H] = wout
    else:
        Atl[H:, H] = wout
    Db = np.zeros((H, H + 1), np.float64)
    Db[:, :H] = np.diag(b)
    Da = np.diag(a)

    return C, {
        "in_Aev": Aev.astype(np.float32),
        "in_Aod": Aod.astype(np.float32),
        "in_Atl": Atl.astype(np.float32),
        "in_Db": Db.astype(np.float32),
        "in_Da": Da.astype(np.float32),
    }


def kernel(x, W_in, b_in, W_hh, W_ih, bias, tau, W_out, b_out):
    C, wmaps = _host_precompute(x, W_in, b_in, W_hh, W_ih, bias, tau,
                                W_out, b_out)
    b_out = np.asarray(b_out, dtype=np.float32)

    if "nc" not in _cached:
        _cached["nc"] = _build_program()
    nc = _cached["nc"]

    in_maps = []
    for i in range(N_CORES):
        C_core = np.ascontiguousarray(
            C[i * B:(i + 1) * B].transpose(1, 2, 0))     # [S, H, B]
        in_maps.append({"in_C": C_core, **wmaps})

    core_ids = list(range(N_CORES))
    res = run_bass_kernel_spmd(nc, in_maps, core_ids)

    out = np.empty((B_FULL, S, 1), dtype=np.float32)
    for i in range(N_CORES):
        dev = res.results[i]["out"].reshape(S, B)        # [s, b_local]
        out[i * B:(i + 1) * B, :, 0] = dev.T + b_out[0]
    return out


# revision 8
# speedup vs baseline: 13.6029x; 1.2484x over previous
"""Trainium2 Bass kernel for nn_LiquidNeuralNetwork (B=512, S=1024, IN=16, HID=64).

Strategy
--------
The reference integrates dh/dt = (-h + tanh(h) @ W_hh.T + inp + bias) / tau
with RK4 x 4 substeps per timestep (16 sequential tanh+matmul rounds per
step).  At dt = 1/1023 the integration error of far cheaper schemes is orders
of magnitude below f32 rounding noise, so we integrate the same ODE with an
exponential integrator + AB2 extrapolation of the (tiny) tanh coupling term:

    H_s = a*H_{s-1} + b*(c_s + 1.5*g_{s-1} - 0.5*g_{s-2}),
    g_s = W_hh @ tanh(H_s),  a = exp(-dt/tau), b = 1 - a,
    c_s = W_ih @ (W_in x_s + b_in) + bias   (precomputed, hidden-major)

which agrees with the reference to ~6e-6 (the f32 noise floor of the
reference itself) while needing ONE tanh + matmul round per timestep.

On-device layout: hidden on partitions, batch on free dim; batch sharded
8 ways (64 per core).  Per round the PSUM bank accumulates the full affine
update via matmuls only:

    bank_r[0:64]  = diag(b) @ c_r + diag(a) @ hm_r + [Wp;Wm] @ [th_r;th_{r-1}]
    bank_r[64]    = W_out @ th_r          (the per-step scalar output)

with Wp = (1.5*b*W_hh)^T, Wm = (-0.5*b*W_hh)^T.  tanh runs on ACT straight
from PSUM; DVE copies bank->SBUF (h materialization + output-row collection);
everything except ACT->PE->ACT is off the critical path.
"""

import os
import numpy as np

import concourse.bacc as bacc
import concourse.tile as tile
from concourse import mybir
from concourse.bass_utils import run_bass_kernel_spmd

F32 = mybir.dt.float32
H = 64          # hidden
BIN = 16        # input features
B_FULL = 512
S = int(os.environ.get("LNN_S", "1024"))   # harness always runs 1024
N_CORES = 8
B = B_FULL // N_CORES   # 64 per-core batch
SEG = 128 if S % 128 == 0 else S           # output segment length (steps)
N_SEG = S // SEG

TRACE = bool(int(os.environ.get("LNN_TRACE", "0")))
SCHEME = os.environ.get("LNN_SCHEME", "pair")   # "pair" | "e2"

NPAIR = S // 2                 # pair rounds
SEGP = NPAIR if NPAIR <= 256 else 256   # pair-slots per output segment
NSEGP = NPAIR // SEGP

_cached = {}


def _build_program():
    """Build + compile the Bass program (same NEFF for all cores)."""
    nc = bacc.Bacc("TRN2", target_bir_lowering=False, debug=False)

    in_C = nc.dram_tensor("in_C", (S, H, B), F32, kind="ExternalInput").ap()
    in_Aev = nc.dram_tensor("in_Aev", (2 * H, H + 1), F32, kind="ExternalInput").ap()
    in_Aod = nc.dram_tensor("in_Aod", (2 * H, H + 1), F32, kind="ExternalInput").ap()
    in_Atl = nc.dram_tensor("in_Atl", (2 * H, H + 1), F32, kind="ExternalInput").ap()
    in_Db = nc.dram_tensor("in_Db", (H, H + 1), F32, kind="ExternalInput").ap()
    in_Da = nc.dram_tensor("in_Da", (H, H), F32, kind="ExternalInput").ap()
    out_dram = nc.dram_tensor("out", (N_SEG, SEG * B), F32, kind="ExternalOutput").ap()

    TANH = mybir.ActivationFunctionType.Tanh

    with tile.TileContext(nc) as tc:
        with (
            tc.tile_pool(name="wts", bufs=1) as wts,
            tc.tile_pool(name="thp", bufs=1) as thp,
            tc.tile_pool(name="osb", bufs=2) as osbp,
            tc.tile_pool(name="cp", bufs=10) as cp,
            tc.tile_pool(name="hmp", bufs=3) as hmp,
            tc.tile_pool(name="hbank", bufs=4, space="PSUM") as hbank,
        ):
            t_Aev = wts.tile([2 * H, H + 1], F32, tag="aev")
            t_Aod = wts.tile([2 * H, H + 1], F32, tag="aod")
            t_Atl = wts.tile([2 * H, H + 1], F32, tag="atl")
            t_Db = wts.tile([H, H + 1], F32, tag="db")
            t_Da = wts.tile([H, H], F32, tag="da")
            nc.sync.dma_start(out=t_Aev, in_=in_Aev)
            nc.sync.dma_start(out=t_Aod, in_=in_Aod)
            nc.sync.dma_start(out=t_Atl, in_=in_Atl)
            nc.sync.dma_start(out=t_Db, in_=in_Db)
            nc.sync.dma_start(out=t_Da, in_=in_Da)

            # persistent tanh tile: half0 = th of even rounds, half1 = odd
            t_th = thp.tile([2 * H, B], F32, tag="th")
            nc.vector.memset(t_th, 0.0)

            # output staging: only partition 64 is used; slot o at free
            # offset (o % SEG)*B.  Two tiles ping-pong across segments.
            t_osb = [osbp.tile([H + 1, SEG * B], F32, tag="osb", name=f"t_osb{i}")
                     for i in range(2)]

            prev_bank = None
            for r in range(1, S):
                t_c = cp.tile([H, B], F32, tag="c")
                nc.sync.dma_start(out=t_c, in_=in_C[r])

                bank = hbank.tile([H + 1, B], F32, tag="bank")
                last = r == 1
                # M4 first (start=True): clears rows 0..64 (col H of Db is 0)
                nc.tensor.matmul(bank, t_Db, t_c, start=True, stop=last)

                if r >= 2:
                    o = r - 2          # output index evacuated this round
                    seg, slot = divmod(o, SEG)
                    # evacuate prev bank's output row (lane-aligned copy)
                    nc.vector.tensor_copy(
                        t_osb[seg % 2][H:H + 1, slot * B:(slot + 1) * B],
                        prev_bank[H:H + 1, :],
                    )
                    if slot == SEG - 1:
                        nc.sync.dma_start(
                            out=out_dram[seg],
                            in_=t_osb[seg % 2][H:H + 1, :],
                        )
                    # h materialization for the decay term
                    t_hm = hmp.tile([H, B], F32, tag="hm")
                    nc.vector.tensor_copy(t_hm, prev_bank[:H, :])
                    # tanh straight from PSUM into this round's th half
                    half = r % 2
                    nc.scalar.activation(
                        t_th[half * H:(half + 1) * H, :], prev_bank[:H, :], TANH)
                    nc.tensor.matmul(bank[:H, :], t_Da, t_hm,
                                     start=False, stop=False)
                    t_A = t_Aev if r % 2 == 0 else t_Aod
                    nc.tensor.matmul(bank, t_A, t_th, start=False, stop=True)
                prev_bank = bank

            # tail: evacuate out_{S-2}; th_S = tanh(H_{S-1}); out_{S-1}
            o = S - 2
            seg, slot = divmod(o, SEG)
            nc.vector.tensor_copy(
                t_osb[seg % 2][H:H + 1, slot * B:(slot + 1) * B],
                prev_bank[H:H + 1, :],
            )
            half = S % 2
            nc.scalar.activation(
                t_th[half * H:(half + 1) * H, :], prev_bank[:H, :], TANH)
            tbank = hbank.tile([H + 1, B], F32, tag="bank")
            nc.tensor.matmul(tbank, t_Atl, t_th, start=True, stop=True)
            o = S - 1
            seg, slot = divmod(o, SEG)
            nc.vector.tensor_copy(
                t_osb[seg % 2][H:H + 1, slot * B:(slot + 1) * B],
                tbank[H:H + 1, :],
            )
            nc.sync.dma_start(out=out_dram[seg], in_=t_osb[seg % 2][H:H + 1, :])

    nc.compile()
    return nc


def _build_program_pair():
    """Pair-corrected scheme: 2 timesteps per tanh round (S/2 rounds).

    PSUM bank halves = [H_s ; H_{s+1}^pred]; one ACT tanh covers both.
    State propagation is exact AB2 (prediction error of odd tanh inputs is
    corrected one round later via deeper tanh history), validated ~9e-6.
    """
    nc = bacc.Bacc("TRN2", target_bir_lowering=False, debug=False)

    in_C = nc.dram_tensor("in_C", (S, H, B), F32, kind="ExternalInput").ap()
    ins = {}
    for nm in ("LC", "LH", "LT1", "LT2", "LB"):
        ins[nm] = nc.dram_tensor(f"in_{nm}", (2 * H, 2 * H), F32,
                                 kind="ExternalInput").ap()
    ins["LO"] = nc.dram_tensor("in_LO", (2 * H, 2), F32,
                               kind="ExternalInput").ap()
    out_dram = nc.dram_tensor("out", (NSEGP, 2, SEGP * B), F32,
                              kind="ExternalOutput").ap()

    TANH = mybir.ActivationFunctionType.Tanh

    with tile.TileContext(nc) as tc:
        with (
            tc.tile_pool(name="wts", bufs=1) as wts,
            tc.tile_pool(name="thp", bufs=4) as thp,
            tc.tile_pool(name="thz", bufs=1) as thz,
            tc.tile_pool(name="osb", bufs=2) as osbp,
            tc.tile_pool(name="cpr", bufs=8) as cpr,
            tc.tile_pool(name="hmp", bufs=3) as hmp,
            tc.tile_pool(name="hbank", bufs=4, space="PSUM") as hbank,
            tc.tile_pool(name="obank", bufs=3, space="PSUM") as obankp,
        ):
            t_w = {}
            for nm in ("LC", "LH", "LT1", "LT2", "LB"):
                t_w[nm] = wts.tile([2 * H, 2 * H], F32, name=f"t_{nm}")
                nc.sync.dma_start(out=t_w[nm], in_=ins[nm])
            t_w["LO"] = wts.tile([2 * H, 2], F32, name="t_LO")
            nc.sync.dma_start(out=t_w["LO"], in_=ins["LO"])

            t_zero = thz.tile([2 * H, B], F32, tag="t1zero")
            nc.vector.memset(t_zero, 0.0)
            t_osb = [osbp.tile([2, SEGP * B], F32, tag="osb", name=f"t_osb{i}")
                     for i in range(2)]

            # boot: bank_0 = [0 ; b*c_1]
            t_cp = cpr.tile([2 * H, B], F32, tag="cp")
            nc.sync.dma_start(out=t_cp, in_=in_C[0:2].flatten_outer_dims())
            bank = hbank.tile([2 * H, B], F32, tag="bank")
            nc.tensor.matmul(bank, t_w["LB"], t_cp, start=True, stop=True)

            prev_bank = bank
            prev_T1 = t_zero
            prev_ob = None
            for r in range(1, NPAIR):
                t_cp = cpr.tile([2 * H, B], F32, tag="cp")
                nc.sync.dma_start(out=t_cp,
                                  in_=in_C[2 * r:2 * r + 2].flatten_outer_dims())
                bank = hbank.tile([2 * H, B], F32, tag="bank")
                nc.tensor.matmul(bank, t_w["LC"], t_cp, start=True, stop=False)

                # from prev bank: tanh pair + h materialization
                T1 = thp.tile([2 * H, B], F32, tag="t1")
                nc.scalar.activation(T1, prev_bank, TANH)
                t_hm = hmp.tile([2 * H, B], F32, tag="hm")
                nc.vector.tensor_copy(t_hm, prev_bank)

                if prev_ob is not None:
                    m = r - 2
                    seg, slot = divmod(m, SEGP)
                    nc.vector.tensor_copy(
                        t_osb[seg % 2][0:2, slot * B:(slot + 1) * B], prev_ob)
                    if slot == SEGP - 1:
                        nc.sync.dma_start(out=out_dram[seg],
                                          in_=t_osb[seg % 2][0:2, :])

                nc.tensor.matmul(bank, t_w["LH"], t_hm, start=False, stop=False)
                nc.tensor.matmul(bank, t_w["LT2"], prev_T1,
                                 start=False, stop=False)
                nc.tensor.matmul(bank, t_w["LT1"], T1, start=False, stop=True)
                ob = obankp.tile([2, B], F32, tag="ob")
                nc.tensor.matmul(ob, t_w["LO"], T1, start=True, stop=True)

                prev_bank, prev_T1, prev_ob = bank, T1, ob

            # tail: last tanh pair -> outs (S-2, S-1); flush last segment
            T1 = thp.tile([2 * H, B], F32, tag="t1")
            nc.scalar.activation(T1, prev_bank, TANH)
            m = NPAIR - 2
            seg, slot = divmod(m, SEGP)
            nc.vector.tensor_copy(
                t_osb[seg % 2][0:2, slot * B:(slot + 1) * B], prev_ob)
            ob = obankp.tile([2, B], F32, tag="ob")
            nc.tensor.matmul(ob, t_w["LO"], T1, start=True, stop=True)
            m = NPAIR - 1
            seg, slot = divmod(m, SEGP)
            nc.vector.tensor_copy(
                t_osb[seg % 2][0:2, slot * B:(slot + 1) * B], ob)
            nc.sync.dma_start(out=out_dram[seg], in_=t_osb[seg % 2][0:2, :])

    nc.compile()
    return nc


def _pair_weights(a, b, W_hh, W_out):
    """Host lhsT matrices for the pair-corrected scheme (f64 in, f32 out)."""
    W = W_hh.astype(np.float64)
    wout = W_out[0].astype(np.float64)
    ab, a2, a2b = a * b, a * a, a * a * b

    def blk(v):
        return (v[:, None] * W).T

    LC = np.zeros((2 * H, 2 * H))
    LC[:H, :H] = np.diag(b)
    LC[:H, H:] = np.diag(ab)
    LC[H:, H:] = np.diag(b)
    LH = np.zeros((2 * H, 2 * H))
    LH[H:, :H] = np.diag(a)
    LH[H:, H:] = np.diag(a2)
    LT1 = np.zeros((2 * H, 2 * H))
    LT1[:H, :H] = blk(-0.5 * b + 1.5 * ab)
    LT1[:H, H:] = blk(-0.5 * ab + 1.5 * a2b - 1.5 * b)
    LT1[H:, :H] = blk(1.5 * b)
    LT1[H:, H:] = blk(1.5 * ab + 2.5 * b)
    LT2 = np.zeros((2 * H, 2 * H))
    LT2[:H, :H] = blk(1.5 * ab)
    LT2[:H, H:] = blk(1.5 * a2b)
    LT2[H:, :H] = blk(-3.0 * ab)
    LT2[H:, H:] = blk(-3.0 * a2b)
    LB = np.zeros((2 * H, 2 * H))
    LB[H:, H:] = np.diag(b)
    LO = np.zeros((2 * H, 2))
    LO[:H, 0] = wout
    LO[H:, 1] = wout
    return {f"in_{nm}": m.astype(np.float32)
            for nm, m in [("LC", LC), ("LH", LH), ("LT1", LT1),
                          ("LT2", LT2), ("LB", LB), ("LO", LO)]}


def _host_precompute(x, W_in, b_in, W_hh, W_ih, bias, tau, W_out, b_out):
    x = np.asarray(x, dtype=np.float32)
    W_in = np.asarray(W_in, dtype=np.float32)
    b_in = np.asarray(b_in, dtype=np.float32)
    W_hh = np.asarray(W_hh, dtype=np.float32)
    W_ih = np.asarray(W_ih, dtype=np.float32)
    bias = np.asarray(bias, dtype=np.float32)
    tau = np.asarray(tau, dtype=np.float32)
    W_out = np.asarray(W_out, dtype=np.float32)

    W_comb = W_ih @ W_in                      # [H, BIN]
    b_comb = W_ih @ b_in + bias               # [H]
    C = x @ W_comb.T + b_comb                 # [B_FULL, S, H] f32

    t = np.linspace(0.0, 1.0, S).astype(np.float32)
    dt = np.float64(t[1]) - np.float64(t[0])
    d = 1.0 / tau.astype(np.float64)
    a = np.exp(-d * dt)
    b = 1.0 - a

    Wp = (1.5 * b[:, None] * W_hh.astype(np.float64)).T   # lhsT [k, j]
    Wm = (-0.5 * b[:, None] * W_hh.astype(np.float64)).T
    wout = W_out[0].astype(np.float64)                    # [H]

    Aev = np.zeros((2 * H, H + 1), np.float64)
    Aev[:H, :H] = Wp
    Aev[H:, :H] = Wm
    Aev[:H, H] = wout
    Aod = np.zeros((2 * H, H + 1), np.float64)
    Aod[:H, :H] = Wm
    Aod[H:, :H] = Wp
    Aod[H:, H] = wout
    # tail round index S (=1024, even): th_S lives in half S%2
    Atl = np.zeros((2 * H, H + 1), np.float64)
    if S % 2 == 0:
        Atl[:H, H] = wout
    else:
        Atl[H:, H] = wout
    Db = np.zeros((H, H + 1), np.float64)
    Db[:, :H] = np.diag(b)
    Da = np.diag(a)

    return C, {
        "in_Aev": Aev.astype(np.float32),
        "in_Aod": Aod.astype(np.float32),
        "in_Atl": Atl.astype(np.float32),
        "in_Db": Db.astype(np.float32),
        "in_Da": Da.astype(np.float32),
    }


def kernel(x, W_in, b_in, W_hh, W_ih, bias, tau, W_out, b_out):
    C, wmaps = _host_precompute(x, W_in, b_in, W_hh, W_ih, bias, tau,
                                W_out, b_out)
    b_out = np.asarray(b_out, dtype=np.float32)

    if SCHEME == "pair":
        t = np.linspace(0.0, 1.0, S).astype(np.float32)
        dt = np.float64(t[1]) - np.float64(t[0])
        d = 1.0 / np.asarray(tau, dtype=np.float32).astype(np.float64)
        a = np.exp(-d * dt)
        b = 1.0 - a
        wmaps = _pair_weights(a, b, np.asarray(W_hh, np.float32),
                              np.asarray(W_out, np.float32))
        builder = _build_program_pair
    else:
        builder = _build_program

    if "nc" not in _cached:
        _cached["nc"] = builder()
    nc = _cached["nc"]

    in_maps = []
    for i in range(N_CORES):
        C_core = np.ascontiguousarray(
            C[i * B:(i + 1) * B].transpose(1, 2, 0))     # [S, H, B]
        in_maps.append({"in_C": C_core, **wmaps})

    core_ids = list(range(N_CORES))
    _cached["in_maps"] = in_maps
    res = run_bass_kernel_spmd(nc, in_maps, core_ids)

    out = np.empty((B_FULL, S, 1), dtype=np.float32)
    for i in range(N_CORES):
        if SCHEME == "pair":
            dev = res.results[i]["out"].reshape(NSEGP, 2, SEGP, B)
            dev = dev.transpose(0, 2, 1, 3).reshape(S, B)   # [o, b]
        else:
            dev = res.results[i]["out"].reshape(S, B)        # [s, b_local]
        out[i * B:(i + 1) * B, :, 0] = dev.T + b_out[0]
    return out


def _in_maps_for_test(C, wmaps):
    maps = []
    for i in range(N_CORES):
        C_core = np.ascontiguousarray(C[i * B:(i + 1) * B].transpose(1, 2, 0))
        maps.append({"in_C": C_core, **wmaps})
    return maps


# revision 11
# speedup vs baseline: 17.3228x; 1.2735x over previous
"""Trainium2 Bass kernel for nn_LiquidNeuralNetwork (B=512, S=1024, IN=16, HID=64).

Strategy
--------
The reference integrates dh/dt = (-h + tanh(h) @ W_hh.T + inp + bias) / tau
with RK4 x 4 substeps per timestep (16 sequential tanh+matmul rounds per
step).  At dt = 1/1023 the integration error of far cheaper schemes is orders
of magnitude below f32 rounding noise, so we integrate the same ODE with an
exponential integrator + AB2 extrapolation of the (tiny) tanh coupling term:

    H_s = a*H_{s-1} + b*(c_s + 1.5*g_{s-1} - 0.5*g_{s-2}),
    g_s = W_hh @ tanh(H_s),  a = exp(-dt/tau), b = 1 - a,
    c_s = W_ih @ (W_in x_s + b_in) + bias   (precomputed, hidden-major)

which agrees with the reference to ~6e-6 (the f32 noise floor of the
reference itself) while needing ONE tanh + matmul round per timestep.

On-device layout: hidden on partitions, batch on free dim; batch sharded
8 ways (64 per core).  Per round the PSUM bank accumulates the full affine
update via matmuls only:

    bank_r[0:64]  = diag(b) @ c_r + diag(a) @ hm_r + [Wp;Wm] @ [th_r;th_{r-1}]
    bank_r[64]    = W_out @ th_r          (the per-step scalar output)

with Wp = (1.5*b*W_hh)^T, Wm = (-0.5*b*W_hh)^T.  tanh runs on ACT straight
from PSUM; DVE copies bank->SBUF (h materialization + output-row collection);
everything except ACT->PE->ACT is off the critical path.
"""

import os
import numpy as np

import concourse.bacc as bacc
import concourse.tile as tile
from concourse import mybir
from concourse.bass_utils import run_bass_kernel_spmd

F32 = mybir.dt.float32
H = 64          # hidden
BIN = 16        # input features
B_FULL = 512
S = int(os.environ.get("LNN_S", "1024"))   # harness always runs 1024
N_CORES = 8
B = B_FULL // N_CORES   # 64 per-core batch
SEG = 128 if S % 128 == 0 else S           # output segment length (steps)
N_SEG = S // SEG

TRACE = bool(int(os.environ.get("LNN_TRACE", "0")))
SCHEME = os.environ.get("LNN_SCHEME", "pair")   # "pair" | "e2"

NPAIR = S // 2                 # pair rounds
SEGP = NPAIR if NPAIR <= 256 else 256   # pair-slots per output segment
NSEGP = NPAIR // SEGP

_cached = {}


def _build_program():
    """Build + compile the Bass program (same NEFF for all cores)."""
    nc = bacc.Bacc("TRN2", target_bir_lowering=False, debug=False)

    in_C = nc.dram_tensor("in_C", (S, H, B), F32, kind="ExternalInput").ap()
    in_Aev = nc.dram_tensor("in_Aev", (2 * H, H + 1), F32, kind="ExternalInput").ap()
    in_Aod = nc.dram_tensor("in_Aod", (2 * H, H + 1), F32, kind="ExternalInput").ap()
    in_Atl = nc.dram_tensor("in_Atl", (2 * H, H + 1), F32, kind="ExternalInput").ap()
    in_Db = nc.dram_tensor("in_Db", (H, H + 1), F32, kind="ExternalInput").ap()
    in_Da = nc.dram_tensor("in_Da", (H, H), F32, kind="ExternalInput").ap()
    out_dram = nc.dram_tensor("out", (N_SEG, SEG * B), F32, kind="ExternalOutput").ap()

    TANH = mybir.ActivationFunctionType.Tanh

    with tile.TileContext(nc) as tc:
        with (
            tc.tile_pool(name="wts", bufs=1) as wts,
            tc.tile_pool(name="thp", bufs=1) as thp,
            tc.tile_pool(name="osb", bufs=2) as osbp,
            tc.tile_pool(name="cp", bufs=10) as cp,
            tc.tile_pool(name="hmp", bufs=3) as hmp,
            tc.tile_pool(name="hbank", bufs=4, space="PSUM") as hbank,
        ):
            t_Aev = wts.tile([2 * H, H + 1], F32, tag="aev")
            t_Aod = wts.tile([2 * H, H + 1], F32, tag="aod")
            t_Atl = wts.tile([2 * H, H + 1], F32, tag="atl")
            t_Db = wts.tile([H, H + 1], F32, tag="db")
            t_Da = wts.tile([H, H], F32, tag="da")
            nc.sync.dma_start(out=t_Aev, in_=in_Aev)
            nc.sync.dma_start(out=t_Aod, in_=in_Aod)
            nc.sync.dma_start(out=t_Atl, in_=in_Atl)
            nc.sync.dma_start(out=t_Db, in_=in_Db)
            nc.sync.dma_start(out=t_Da, in_=in_Da)

            # persistent tanh tile: half0 = th of even rounds, half1 = odd
            t_th = thp.tile([2 * H, B], F32, tag="th")
            nc.vector.memset(t_th, 0.0)

            # output staging: only partition 64 is used; slot o at free
            # offset (o % SEG)*B.  Two tiles ping-pong across segments.
            t_osb = [osbp.tile([H + 1, SEG * B], F32, tag="osb", name=f"t_osb{i}")
                     for i in range(2)]

            prev_bank = None
            for r in range(1, S):
                t_c = cp.tile([H, B], F32, tag="c")
                nc.sync.dma_start(out=t_c, in_=in_C[r])

                bank = hbank.tile([H + 1, B], F32, tag="bank")
                last = r == 1
                # M4 first (start=True): clears rows 0..64 (col H of Db is 0)
                nc.tensor.matmul(bank, t_Db, t_c, start=True, stop=last)

                if r >= 2:
                    o = r - 2          # output index evacuated this round
                    seg, slot = divmod(o, SEG)
                    # evacuate prev bank's output row (lane-aligned copy)
                    nc.vector.tensor_copy(
                        t_osb[seg % 2][H:H + 1, slot * B:(slot + 1) * B],
                        prev_bank[H:H + 1, :],
                    )
                    if slot == SEG - 1:
                        nc.sync.dma_start(
                            out=out_dram[seg],
                            in_=t_osb[seg % 2][H:H + 1, :],
                        )
                    # h materialization for the decay term
                    t_hm = hmp.tile([H, B], F32, tag="hm")
                    nc.vector.tensor_copy(t_hm, prev_bank[:H, :])
                    # tanh straight from PSUM into this round's th half
                    half = r % 2
                    nc.scalar.activation(
                        t_th[half * H:(half + 1) * H, :], prev_bank[:H, :], TANH)
                    nc.tensor.matmul(bank[:H, :], t_Da, t_hm,
                                     start=False, stop=False)
                    t_A = t_Aev if r % 2 == 0 else t_Aod
                    nc.tensor.matmul(bank, t_A, t_th, start=False, stop=True)
                prev_bank = bank

            # tail: evacuate out_{S-2}; th_S = tanh(H_{S-1}); out_{S-1}
            o = S - 2
            seg, slot = divmod(o, SEG)
            nc.vector.tensor_copy(
                t_osb[seg % 2][H:H + 1, slot * B:(slot + 1) * B],
                prev_bank[H:H + 1, :],
            )
            half = S % 2
            nc.scalar.activation(
                t_th[half * H:(half + 1) * H, :], prev_bank[:H, :], TANH)
            tbank = hbank.tile([H + 1, B], F32, tag="bank")
            nc.tensor.matmul(tbank, t_Atl, t_th, start=True, stop=True)
            o = S - 1
            seg, slot = divmod(o, SEG)
            nc.vector.tensor_copy(
                t_osb[seg % 2][H:H + 1, slot * B:(slot + 1) * B],
                tbank[H:H + 1, :],
            )
            nc.sync.dma_start(out=out_dram[seg], in_=t_osb[seg % 2][H:H + 1, :])

    nc.compile()
    return nc


def _build_program_pair():
    """Pair-corrected scheme v2: 2 timesteps per tanh round (S/2 rounds).

    PSUM bank halves = [H_s ; H_{s+1}^pred]; one bf16 ACT tanh covers both
    and feeds the (tiny) tanh-coupling matmuls LT1/LT2 in bf16; a second f32
    tanh feeds the f32 output matvec.  The c-injection is folded into the
    f32 decay matmul LH via a host-prescaled C'' tile DMA'd into the hm
    tile, whose lower half gets H_{s-1} added by one DVE op:
        hm = [b*c_{s+1} ; (b/a)*c_s + H_{s-1}]
        LH @ hm = [a*H_{s-1}+b*c_s ; a^2*H_{s-1}+ab*c_s+b*c_{s+1}]
    """
    nc = bacc.Bacc("TRN2", target_bir_lowering=False, debug=False)

    BF16 = mybir.dt.bfloat16
    GDT = BF16 if os.environ.get("LNN_GDT", "bf16") == "bf16" else F32

    in_C = nc.dram_tensor("in_C", (NPAIR, 2 * H, B), F32,
                          kind="ExternalInput").ap()
    ins = {}
    for nm in ("LH", "LB"):
        ins[nm] = nc.dram_tensor(f"in_{nm}", (2 * H, 2 * H), F32,
                                 kind="ExternalInput").ap()
    for nm in ("LT1", "LT2"):
        ins[nm] = nc.dram_tensor(f"in_{nm}", (2 * H, 2 * H), GDT,
                                 kind="ExternalInput").ap()
    ins["LO"] = nc.dram_tensor("in_LO", (2 * H, 2), F32,
                               kind="ExternalInput").ap()
    out_dram = nc.dram_tensor("out", (NSEGP, 2, SEGP * B), F32,
                              kind="ExternalOutput").ap()

    TANH = mybir.ActivationFunctionType.Tanh

    with tile.TileContext(nc) as tc:
        with (
            tc.tile_pool(name="wts", bufs=1) as wts,
            tc.tile_pool(name="thp", bufs=4) as thp,
            tc.tile_pool(name="thf", bufs=3) as thfp,
            tc.tile_pool(name="thz", bufs=1) as thz,
            tc.tile_pool(name="osb", bufs=2) as osbp,
            tc.tile_pool(name="hmp", bufs=8) as hmp,
            tc.tile_pool(name="hbank", bufs=4, space="PSUM") as hbank,
            tc.tile_pool(name="obank", bufs=3, space="PSUM") as obankp,
        ):
            t_w = {}
            for nm in ("LH", "LB"):
                t_w[nm] = wts.tile([2 * H, 2 * H], F32, name=f"t_{nm}")
                nc.sync.dma_start(out=t_w[nm], in_=ins[nm])
            for nm in ("LT1", "LT2"):
                t_w[nm] = wts.tile([2 * H, 2 * H], GDT, name=f"t_{nm}")
                nc.sync.dma_start(out=t_w[nm], in_=ins[nm])
            t_w["LO"] = wts.tile([2 * H, 2], F32, name="t_LO")
            nc.sync.dma_start(out=t_w["LO"], in_=ins["LO"])

            t_zero = thz.tile([2 * H, B], GDT, tag="t1zero")
            nc.vector.memset(t_zero, 0.0)
            t_osb = [osbp.tile([2, SEGP * B], F32, tag="osb", name=f"t_osb{i}")
                     for i in range(2)]

            # boot: bank_0 = [0 ; b*c_1]  (C''_0 half0 = b*c_1)
            t_hm = hmp.tile([2 * H, B], F32, tag="hm")
            nc.sync.dma_start(out=t_hm, in_=in_C[0])
            bank = hbank.tile([2 * H, B], F32, tag="bank")
            nc.tensor.matmul(bank, t_w["LB"], t_hm, start=True, stop=True)

            prev_bank = bank
            prev_T1 = t_zero
            prev_ob = None
            for r in range(1, NPAIR):
                t_hm = hmp.tile([2 * H, B], F32, tag="hm")
                nc.sync.dma_start(out=t_hm, in_=in_C[r])

                bank = hbank.tile([2 * H, B], F32, tag="bank")
                nc.tensor.matmul(bank, t_w["LT2"], prev_T1,
                                 start=True, stop=False)

                # tanh pair: bf16 for the coupling path (critical), f32 for
                # the output matvec (off critical path)
                T1 = thp.tile([2 * H, B], GDT, tag="t1")
                nc.scalar.activation(T1, prev_bank, TANH)
                t_thf = thfp.tile([2 * H, B], F32, tag="thf")
                nc.scalar.activation(t_thf, prev_bank, TANH)

                # hm lower half += H_{s-1} (from prev bank)
                nc.vector.tensor_add(t_hm[H:, :], t_hm[H:, :],
                                     prev_bank[H:, :])

                if prev_ob is not None:
                    m = r - 2
                    seg, slot = divmod(m, SEGP)
                    nc.vector.tensor_copy(
                        t_osb[seg % 2][0:2, slot * B:(slot + 1) * B], prev_ob)
                    if slot == SEGP - 1:
                        nc.sync.dma_start(out=out_dram[seg],
                                          in_=t_osb[seg % 2][0:2, :])

                nc.tensor.matmul(bank, t_w["LH"], t_hm, start=False,
                                 stop=False)
                nc.tensor.matmul(bank, t_w["LT1"], T1, start=False, stop=True)
                ob = obankp.tile([2, B], F32, tag="ob")
                nc.tensor.matmul(ob, t_w["LO"], t_thf, start=True, stop=True)

                prev_bank, prev_T1, prev_ob = bank, T1, ob

            # tail: last tanh pair -> outs (S-2, S-1); flush last segment
            t_thf = thfp.tile([2 * H, B], F32, tag="thf")
            nc.scalar.activation(t_thf, prev_bank, TANH)
            m = NPAIR - 2
            seg, slot = divmod(m, SEGP)
            nc.vector.tensor_copy(
                t_osb[seg % 2][0:2, slot * B:(slot + 1) * B], prev_ob)
            ob = obankp.tile([2, B], F32, tag="ob")
            nc.tensor.matmul(ob, t_w["LO"], t_thf, start=True, stop=True)
            m = NPAIR - 1
            seg, slot = divmod(m, SEGP)
            nc.vector.tensor_copy(
                t_osb[seg % 2][0:2, slot * B:(slot + 1) * B], ob)
            nc.sync.dma_start(out=out_dram[seg], in_=t_osb[seg % 2][0:2, :])

    nc.compile()
    return nc



def _pair_weights(a, b, W_hh, W_out):
    """Host lhsT matrices for the pair-corrected scheme (f64 in)."""
    import ml_dtypes
    gdt = (ml_dtypes.bfloat16 if os.environ.get("LNN_GDT", "bf16") == "bf16"
           else np.float32)
    W = W_hh.astype(np.float64)
    wout = W_out[0].astype(np.float64)
    ab, a2, a2b = a * b, a * a, a * a * b

    def blk(v):
        return (v[:, None] * W).T

    LH = np.zeros((2 * H, 2 * H))
    LH[:H, H:] = np.eye(H)
    LH[H:, :H] = np.diag(a)
    LH[H:, H:] = np.diag(a2)
    LT1 = np.zeros((2 * H, 2 * H))
    LT1[:H, :H] = blk(-0.5 * b + 1.5 * ab)
    LT1[:H, H:] = blk(-0.5 * ab + 1.5 * a2b - 1.5 * b)
    LT1[H:, :H] = blk(1.5 * b)
    LT1[H:, H:] = blk(1.5 * ab + 2.5 * b)
    LT2 = np.zeros((2 * H, 2 * H))
    LT2[:H, :H] = blk(1.5 * ab)
    LT2[:H, H:] = blk(1.5 * a2b)
    LT2[H:, :H] = blk(-3.0 * ab)
    LT2[H:, H:] = blk(-3.0 * a2b)
    LB = np.zeros((2 * H, 2 * H))
    LB[:H, H:] = np.eye(H)
    LO = np.zeros((2 * H, 2))
    LO[:H, 0] = wout
    LO[H:, 1] = wout
    return {"in_LH": LH.astype(np.float32),
            "in_LB": LB.astype(np.float32),
            "in_LT1": LT1.astype(gdt),
            "in_LT2": LT2.astype(gdt),
            "in_LO": LO.astype(np.float32)}



def _host_precompute(x, W_in, b_in, W_hh, W_ih, bias, tau, W_out, b_out):
    x = np.asarray(x, dtype=np.float32)
    W_in = np.asarray(W_in, dtype=np.float32)
    b_in = np.asarray(b_in, dtype=np.float32)
    W_hh = np.asarray(W_hh, dtype=np.float32)
    W_ih = np.asarray(W_ih, dtype=np.float32)
    bias = np.asarray(bias, dtype=np.float32)
    tau = np.asarray(tau, dtype=np.float32)
    W_out = np.asarray(W_out, dtype=np.float32)

    W_comb = W_ih @ W_in                      # [H, BIN]
    b_comb = W_ih @ b_in + bias               # [H]
    C = x @ W_comb.T + b_comb                 # [B_FULL, S, H] f32

    t = np.linspace(0.0, 1.0, S).astype(np.float32)
    dt = np.float64(t[1]) - np.float64(t[0])
    d = 1.0 / tau.astype(np.float64)
    a = np.exp(-d * dt)
    b = 1.0 - a

    Wp = (1.5 * b[:, None] * W_hh.astype(np.float64)).T   # lhsT [k, j]
    Wm = (-0.5 * b[:, None] * W_hh.astype(np.float64)).T
    wout = W_out[0].astype(np.float64)                    # [H]

    Aev = np.zeros((2 * H, H + 1), np.float64)
    Aev[:H, :H] = Wp
    Aev[H:, :H] = Wm
    Aev[:H, H] = wout
    Aod = np.zeros((2 * H, H + 1), np.float64)
    Aod[:H, :H] = Wm
    Aod[H:, :H] = Wp
    Aod[H:, H] = wout
    # tail round index S (=1024, even): th_S lives in half S%2
    Atl = np.zeros((2 * H, H + 1), np.float64)
    if S % 2 == 0:
        Atl[:H, H] = wout
    else:
        Atl[H:, H] = wout
    Db = np.zeros((H, H + 1), np.float64)
    Db[:, :H] = np.diag(b)
    Da = np.diag(a)

    return C, {
        "in_Aev": Aev.astype(np.float32),
        "in_Aod": Aod.astype(np.float32),
        "in_Atl": Atl.astype(np.float32),
        "in_Db": Db.astype(np.float32),
        "in_Da": Da.astype(np.float32),
    }


def kernel(x, W_in, b_in, W_hh, W_ih, bias, tau, W_out, b_out):
    C, wmaps = _host_precompute(x, W_in, b_in, W_hh, W_ih, bias, tau,
                                W_out, b_out)
    b_out = np.asarray(b_out, dtype=np.float32)

    if SCHEME == "pair":
        t = np.linspace(0.0, 1.0, S).astype(np.float32)
        dt = np.float64(t[1]) - np.float64(t[0])
        d = 1.0 / np.asarray(tau, dtype=np.float32).astype(np.float64)
        a = np.exp(-d * dt)
        b = 1.0 - a
        wmaps = _pair_weights(a, b, np.asarray(W_hh, np.float32),
                              np.asarray(W_out, np.float32))
        builder = _build_program_pair
        # prescaled pair C'': tile r = [b*c_{2r+1} ; (b/a)*c_{2r}]
        bf = b.astype(np.float32)[None, :]
        baf = (b / a).astype(np.float32)[None, :]
    else:
        builder = _build_program

    if "nc" not in _cached:
        _cached["nc"] = builder()
    nc = _cached["nc"]

    in_maps = []
    for i in range(N_CORES):
        Cc = C[i * B:(i + 1) * B]                        # [B, S, H]
        if SCHEME == "pair":
            odd = (Cc[:, 1::2, :] * bf).transpose(1, 2, 0)   # [NPAIR, H, B]
            even = (Cc[:, 0::2, :] * baf).transpose(1, 2, 0)
            C_core = np.ascontiguousarray(
                np.concatenate([odd, even], axis=1))     # [NPAIR, 2H, B]
        else:
            C_core = np.ascontiguousarray(Cc.transpose(1, 2, 0))  # [S, H, B]
        in_maps.append({"in_C": C_core, **wmaps})

    core_ids = list(range(N_CORES))
    _cached["in_maps"] = in_maps
    res = run_bass_kernel_spmd(nc, in_maps, core_ids)

    out = np.empty((B_FULL, S, 1), dtype=np.float32)
    for i in range(N_CORES):
        if SCHEME == "pair":
            dev = res.results[i]["out"].reshape(NSEGP, 2, SEGP, B)
            dev = dev.transpose(0, 2, 1, 3).reshape(S, B)   # [o, b]
        else:
            dev = res.results[i]["out"].reshape(S, B)        # [s, b_local]
        out[i * B:(i + 1) * B, :, 0] = dev.T + b_out[0]
    return out


def _in_maps_for_test(C, wmaps):
    maps = []
    for i in range(N_CORES):
        C_core = np.ascontiguousarray(C[i * B:(i + 1) * B].transpose(1, 2, 0))
        maps.append({"in_C": C_core, **wmaps})
    return maps


# revision 13
# speedup vs baseline: 17.3832x; 1.0035x over previous
"""Trainium2 Bass kernel for nn_LiquidNeuralNetwork (B=512, S=1024, IN=16, HID=64).

Strategy
--------
The reference integrates dh/dt = (-h + tanh(h) @ W_hh.T + inp + bias) / tau
with RK4 x 4 substeps per timestep (16 sequential tanh+matmul rounds per
step).  At dt = 1/1023 the integration error of far cheaper schemes is orders
of magnitude below f32 rounding noise, so we integrate the same ODE with an
exponential integrator + AB2 extrapolation of the (tiny) tanh coupling term:

    H_s = a*H_{s-1} + b*(c_s + 1.5*g_{s-1} - 0.5*g_{s-2}),
    g_s = W_hh @ tanh(H_s),  a = exp(-dt/tau), b = 1 - a,
    c_s = W_ih @ (W_in x_s + b_in) + bias   (precomputed, hidden-major)

which agrees with the reference to ~6e-6 (the f32 noise floor of the
reference itself) while needing ONE tanh + matmul round per timestep.

On-device layout: hidden on partitions, batch on free dim; batch sharded
8 ways (64 per core).  Per round the PSUM bank accumulates the full affine
update via matmuls only:

    bank_r[0:64]  = diag(b) @ c_r + diag(a) @ hm_r + [Wp;Wm] @ [th_r;th_{r-1}]
    bank_r[64]    = W_out @ th_r          (the per-step scalar output)

with Wp = (1.5*b*W_hh)^T, Wm = (-0.5*b*W_hh)^T.  tanh runs on ACT straight
from PSUM; DVE copies bank->SBUF (h materialization + output-row collection);
everything except ACT->PE->ACT is off the critical path.
"""

import os
import numpy as np

import concourse.bacc as bacc
import concourse.tile as tile
from concourse import mybir
from concourse.bass_utils import run_bass_kernel_spmd

F32 = mybir.dt.float32
H = 64          # hidden
BIN = 16        # input features
B_FULL = 512
S = int(os.environ.get("LNN_S", "1024"))   # harness always runs 1024
N_CORES = 8
B = B_FULL // N_CORES   # 64 per-core batch
SEG = 128 if S % 128 == 0 else S           # output segment length (steps)
N_SEG = S // SEG

TRACE = bool(int(os.environ.get("LNN_TRACE", "0")))
SCHEME = os.environ.get("LNN_SCHEME", "pair")   # "pair" | "e2"

NPAIR = S // 2                 # pair rounds
SEGP = NPAIR if NPAIR <= 256 else 256   # pair-slots per output segment
NSEGP = NPAIR // SEGP

_cached = {}


def _build_program():
    """Build + compile the Bass program (same NEFF for all cores)."""
    nc = bacc.Bacc("TRN2", target_bir_lowering=False, debug=False)

    in_C = nc.dram_tensor("in_C", (S, H, B), F32, kind="ExternalInput").ap()
    in_Aev = nc.dram_tensor("in_Aev", (2 * H, H + 1), F32, kind="ExternalInput").ap()
    in_Aod = nc.dram_tensor("in_Aod", (2 * H, H + 1), F32, kind="ExternalInput").ap()
    in_Atl = nc.dram_tensor("in_Atl", (2 * H, H + 1), F32, kind="ExternalInput").ap()
    in_Db = nc.dram_tensor("in_Db", (H, H + 1), F32, kind="ExternalInput").ap()
    in_Da = nc.dram_tensor("in_Da", (H, H), F32, kind="ExternalInput").ap()
    out_dram = nc.dram_tensor("out", (N_SEG, SEG * B), F32, kind="ExternalOutput").ap()

    TANH = mybir.ActivationFunctionType.Tanh

    with tile.TileContext(nc) as tc:
        with (
            tc.tile_pool(name="wts", bufs=1) as wts,
            tc.tile_pool(name="thp", bufs=1) as thp,
            tc.tile_pool(name="osb", bufs=2) as osbp,
            tc.tile_pool(name="cp", bufs=10) as cp,
            tc.tile_pool(name="hmp", bufs=3) as hmp,
            tc.tile_pool(name="hbank", bufs=4, space="PSUM") as hbank,
        ):
            t_Aev = wts.tile([2 * H, H + 1], F32, tag="aev")
            t_Aod = wts.tile([2 * H, H + 1], F32, tag="aod")
            t_Atl = wts.tile([2 * H, H + 1], F32, tag="atl")
            t_Db = wts.tile([H, H + 1], F32, tag="db")
            t_Da = wts.tile([H, H], F32, tag="da")
            nc.sync.dma_start(out=t_Aev, in_=in_Aev)
            nc.sync.dma_start(out=t_Aod, in_=in_Aod)
            nc.sync.dma_start(out=t_Atl, in_=in_Atl)
            nc.sync.dma_start(out=t_Db, in_=in_Db)
            nc.sync.dma_start(out=t_Da, in_=in_Da)

            # persistent tanh tile: half0 = th of even rounds, half1 = odd
            t_th = thp.tile([2 * H, B], F32, tag="th")
            nc.vector.memset(t_th, 0.0)

            # output staging: only partition 64 is used; slot o at free
            # offset (o % SEG)*B.  Two tiles ping-pong across segments.
            t_osb = [osbp.tile([H + 1, SEG * B], F32, tag="osb", name=f"t_osb{i}")
                     for i in range(2)]

            prev_bank = None
            for r in range(1, S):
                t_c = cp.tile([H, B], F32, tag="c")
                nc.sync.dma_start(out=t_c, in_=in_C[r])

                bank = hbank.tile([H + 1, B], F32, tag="bank")
                last = r == 1
                # M4 first (start=True): clears rows 0..64 (col H of Db is 0)
                nc.tensor.matmul(bank, t_Db, t_c, start=True, stop=last)

                if r >= 2:
                    o = r - 2          # output index evacuated this round
                    seg, slot = divmod(o, SEG)
                    # evacuate prev bank's output row (lane-aligned copy)
                    nc.vector.tensor_copy(
                        t_osb[seg % 2][H:H + 1, slot * B:(slot + 1) * B],
                        prev_bank[H:H + 1, :],
                    )
                    if slot == SEG - 1:
                        nc.sync.dma_start(
                            out=out_dram[seg],
                            in_=t_osb[seg % 2][H:H + 1, :],
                        )
                    # h materialization for the decay term
                    t_hm = hmp.tile([H, B], F32, tag="hm")
                    nc.vector.tensor_copy(t_hm, prev_bank[:H, :])
                    # tanh straight from PSUM into this round's th half
                    half = r % 2
                    nc.scalar.activation(
                        t_th[half * H:(half + 1) * H, :], prev_bank[:H, :], TANH)
                    nc.tensor.matmul(bank[:H, :], t_Da, t_hm,
                                     start=False, stop=False)
                    t_A = t_Aev if r % 2 == 0 else t_Aod
                    nc.tensor.matmul(bank, t_A, t_th, start=False, stop=True)
                prev_bank = bank

            # tail: evacuate out_{S-2}; th_S = tanh(H_{S-1}); out_{S-1}
            o = S - 2
            seg, slot = divmod(o, SEG)
            nc.vector.tensor_copy(
                t_osb[seg % 2][H:H + 1, slot * B:(slot + 1) * B],
                prev_bank[H:H + 1, :],
            )
            half = S % 2
            nc.scalar.activation(
                t_th[half * H:(half + 1) * H, :], prev_bank[:H, :], TANH)
            tbank = hbank.tile([H + 1, B], F32, tag="bank")
            nc.tensor.matmul(tbank, t_Atl, t_th, start=True, stop=True)
            o = S - 1
            seg, slot = divmod(o, SEG)
            nc.vector.tensor_copy(
                t_osb[seg % 2][H:H + 1, slot * B:(slot + 1) * B],
                tbank[H:H + 1, :],
            )
            nc.sync.dma_start(out=out_dram[seg], in_=t_osb[seg % 2][H:H + 1, :])

    nc.compile()
    return nc


def _build_program_pair():
    """Pair-corrected scheme v2: 2 timesteps per tanh round (S/2 rounds).

    PSUM bank halves = [H_s ; H_{s+1}^pred]; one bf16 ACT tanh covers both
    and feeds the (tiny) tanh-coupling matmuls LT1/LT2 in bf16; a second f32
    tanh feeds the f32 output matvec.  The c-injection is folded into the
    f32 decay matmul LH via a host-prescaled C'' tile DMA'd into the hm
    tile, whose lower half gets H_{s-1} added by one DVE op:
        hm = [b*c_{s+1} ; (b/a)*c_s + H_{s-1}]
        LH @ hm = [a*H_{s-1}+b*c_s ; a^2*H_{s-1}+ab*c_s+b*c_{s+1}]
    """
    nc = bacc.Bacc("TRN2", target_bir_lowering=False, debug=False)

    BF16 = mybir.dt.bfloat16
    GDT = BF16 if os.environ.get("LNN_GDT", "bf16") == "bf16" else F32

    in_C = nc.dram_tensor("in_C", (NPAIR, 2 * H, B), F32,
                          kind="ExternalInput").ap()
    ins = {}
    for nm in ("LH", "LB"):
        ins[nm] = nc.dram_tensor(f"in_{nm}", (2 * H, 2 * H), F32,
                                 kind="ExternalInput").ap()
    for nm in ("LT1", "LT2"):
        ins[nm] = nc.dram_tensor(f"in_{nm}", (2 * H, 2 * H), GDT,
                                 kind="ExternalInput").ap()
    ins["LO"] = nc.dram_tensor("in_LO", (2 * H, 2), F32,
                               kind="ExternalInput").ap()
    out_dram = nc.dram_tensor("out", (NSEGP, 2, SEGP * B), F32,
                              kind="ExternalOutput").ap()

    TANH = mybir.ActivationFunctionType.Tanh

    with tile.TileContext(nc) as tc:
        with (
            tc.tile_pool(name="wts", bufs=1) as wts,
            tc.tile_pool(name="thp", bufs=4) as thp,
            tc.tile_pool(name="thf", bufs=3) as thfp,
            tc.tile_pool(name="thz", bufs=1) as thz,
            tc.tile_pool(name="osb", bufs=2) as osbp,
            tc.tile_pool(name="hmp", bufs=8) as hmp,
            tc.tile_pool(name="hbank", bufs=4, space="PSUM") as hbank,
            tc.tile_pool(name="obank", bufs=3, space="PSUM") as obankp,
        ):
            t_w = {}
            for nm in ("LH", "LB"):
                t_w[nm] = wts.tile([2 * H, 2 * H], F32, name=f"t_{nm}")
                nc.sync.dma_start(out=t_w[nm], in_=ins[nm])
            for nm in ("LT1", "LT2"):
                t_w[nm] = wts.tile([2 * H, 2 * H], GDT, name=f"t_{nm}")
                nc.sync.dma_start(out=t_w[nm], in_=ins[nm])
            t_w["LO"] = wts.tile([2 * H, 2], F32, name="t_LO")
            nc.sync.dma_start(out=t_w["LO"], in_=ins["LO"])

            t_zero = thz.tile([2 * H, B], GDT, tag="t1zero")
            nc.vector.memset(t_zero, 0.0)
            t_osb = [osbp.tile([2, SEGP * B], F32, tag="osb", name=f"t_osb{i}")
                     for i in range(2)]

            # boot: bank_0 = [0 ; b*c_1]  (C''_0 half0 = b*c_1)
            t_hm = hmp.tile([2 * H, B], F32, tag="hm")
            nc.sync.dma_start(out=t_hm, in_=in_C[0])
            bank = hbank.tile([2 * H, B], F32, tag="bank")
            nc.tensor.matmul(bank, t_w["LB"], t_hm, start=True, stop=True)

            prev_bank = bank
            prev_T1 = t_zero
            prev_thf = None           # f32 tanh pair awaiting its out matvec
            pending = []              # [(ob_tile, slot_index)] not yet evac'd

            def flush_one():
                ob_t, m = pending.pop(0)
                seg, slot = divmod(m, SEGP)
                nc.vector.tensor_copy(
                    t_osb[seg % 2][0:2, slot * B:(slot + 1) * B], ob_t)
                if slot == SEGP - 1:
                    nc.sync.dma_start(out=out_dram[seg],
                                      in_=t_osb[seg % 2][0:2, :])

            for r in range(1, NPAIR):
                t_hm = hmp.tile([2 * H, B], F32, tag="hm")
                nc.sync.dma_start(out=t_hm, in_=in_C[r])

                bank = hbank.tile([2 * H, B], F32, tag="bank")
                # bf16 matmul first (FWL-friendly after last round's bf16 LT1)
                nc.tensor.matmul(bank, t_w["LT2"], prev_T1,
                                 start=True, stop=False)
                # the two f32 matmuls adjacent: previous round's out matvec,
                # then the decay+input injection
                if prev_thf is not None:
                    ob = obankp.tile([2, B], F32, tag="ob")
                    nc.tensor.matmul(ob, t_w["LO"], prev_thf,
                                     start=True, stop=True)
                    pending.append((ob, r - 2))

                # tanh pair: bf16 for the coupling path (critical), f32 for
                # the output matvec (off critical path)
                T1 = thp.tile([2 * H, B], GDT, tag="t1")
                nc.scalar.activation(T1, prev_bank, TANH)
                t_thf = thfp.tile([2 * H, B], F32, tag="thf")
                nc.scalar.activation(t_thf, prev_bank, TANH)

                # hm lower half += H_{s-1} (from prev bank)
                nc.vector.tensor_add(t_hm[H:, :], t_hm[H:, :],
                                     prev_bank[H:, :])

                if len(pending) > 1:
                    flush_one()

                nc.tensor.matmul(bank, t_w["LH"], t_hm, start=False,
                                 stop=False)
                nc.tensor.matmul(bank, t_w["LT1"], T1, start=False, stop=True)

                prev_bank, prev_T1, prev_thf = bank, T1, t_thf

            # tail: emit out matvecs for the last two tanh pairs, flush all
            ob = obankp.tile([2, B], F32, tag="ob")
            nc.tensor.matmul(ob, t_w["LO"], prev_thf, start=True, stop=True)
            pending.append((ob, NPAIR - 2))
            t_thf = thfp.tile([2 * H, B], F32, tag="thf")
            nc.scalar.activation(t_thf, prev_bank, TANH)
            ob = obankp.tile([2, B], F32, tag="ob")
            nc.tensor.matmul(ob, t_w["LO"], t_thf, start=True, stop=True)
            pending.append((ob, NPAIR - 1))
            while pending:
                flush_one()   # final segment's DMA fires on its last slot

    nc.compile()
    return nc



def _pair_weights(a, b, W_hh, W_out):
    """Host lhsT matrices for the pair-corrected scheme (f64 in)."""
    import ml_dtypes
    gdt = (ml_dtypes.bfloat16 if os.environ.get("LNN_GDT", "bf16") == "bf16"
           else np.float32)
    W = W_hh.astype(np.float64)
    wout = W_out[0].astype(np.float64)
    ab, a2, a2b = a * b, a * a, a * a * b

    def blk(v):
        return (v[:, None] * W).T

    LH = np.zeros((2 * H, 2 * H))
    LH[:H, H:] = np.eye(H)
    LH[H:, :H] = np.diag(a)
    LH[H:, H:] = np.diag(a2)
    LT1 = np.zeros((2 * H, 2 * H))
    LT1[:H, :H] = blk(-0.5 * b + 1.5 * ab)
    LT1[:H, H:] = blk(-0.5 * ab + 1.5 * a2b - 1.5 * b)
    LT1[H:, :H] = blk(1.5 * b)
    LT1[H:, H:] = blk(1.5 * ab + 2.5 * b)
    LT2 = np.zeros((2 * H, 2 * H))
    LT2[:H, :H] = blk(1.5 * ab)
    LT2[:H, H:] = blk(1.5 * a2b)
    LT2[H:, :H] = blk(-3.0 * ab)
    LT2[H:, H:] = blk(-3.0 * a2b)
    LB = np.zeros((2 * H, 2 * H))
    LB[:H, H:] = np.eye(H)
    LO = np.zeros((2 * H, 2))
    LO[:H, 0] = wout
    LO[H:, 1] = wout
    return {"in_LH": LH.astype(np.float32),
            "in_LB": LB.astype(np.float32),
            "in_LT1": LT1.astype(gdt),
            "in_LT2": LT2.astype(gdt),
            "in_LO": LO.astype(np.float32)}



def _host_precompute(x, W_in, b_in, W_hh, W_ih, bias, tau, W_out, b_out):
    x = np.asarray(x, dtype=np.float32)
    W_in = np.asarray(W_in, dtype=np.float32)
    b_in = np.asarray(b_in, dtype=np.float32)
    W_hh = np.asarray(W_hh, dtype=np.float32)
    W_ih = np.asarray(W_ih, dtype=np.float32)
    bias = np.asarray(bias, dtype=np.float32)
    tau = np.asarray(tau, dtype=np.float32)
    W_out = np.asarray(W_out, dtype=np.float32)

    W_comb = W_ih @ W_in                      # [H, BIN]
    b_comb = W_ih @ b_in + bias               # [H]
    C = x @ W_comb.T + b_comb                 # [B_FULL, S, H] f32

    t = np.linspace(0.0, 1.0, S).astype(np.float32)
    dt = np.float64(t[1]) - np.float64(t[0])
    d = 1.0 / tau.astype(np.float64)
    a = np.exp(-d * dt)
    b = 1.0 - a

    Wp = (1.5 * b[:, None] * W_hh.astype(np.float64)).T   # lhsT [k, j]
    Wm = (-0.5 * b[:, None] * W_hh.astype(np.float64)).T
    wout = W_out[0].astype(np.float64)                    # [H]

    Aev = np.zeros((2 * H, H + 1), np.float64)
    Aev[:H, :H] = Wp
    Aev[H:, :H] = Wm
    Aev[:H, H] = wout
    Aod = np.zeros((2 * H, H + 1), np.float64)
    Aod[:H, :H] = Wm
    Aod[H:, :H] = Wp
    Aod[H:, H] = wout
    # tail round index S (=1024, even): th_S lives in half S%2
    Atl = np.zeros((2 * H, H + 1), np.float64)
    if S % 2 == 0:
        Atl[:H, H] = wout
    else:
        Atl[H:, H] = wout
    Db = np.zeros((H, H + 1), np.float64)
    Db[:, :H] = np.diag(b)
    Da = np.diag(a)

    return C, {
        "in_Aev": Aev.astype(np.float32),
        "in_Aod": Aod.astype(np.float32),
        "in_Atl": Atl.astype(np.float32),
        "in_Db": Db.astype(np.float32),
        "in_Da": Da.astype(np.float32),
    }


def kernel(x, W_in, b_in, W_hh, W_ih, bias, tau, W_out, b_out):
    C, wmaps = _host_precompute(x, W_in, b_in, W_hh, W_ih, bias, tau,
                                W_out, b_out)
    b_out = np.asarray(b_out, dtype=np.float32)

    if SCHEME == "pair":
        t = np.linspace(0.0, 1.0, S).astype(np.float32)
        dt = np.float64(t[1]) - np.float64(t[0])
        d = 1.0 / np.asarray(tau, dtype=np.float32).astype(np.float64)
        a = np.exp(-d * dt)
        b = 1.0 - a
        wmaps = _pair_weights(a, b, np.asarray(W_hh, np.float32),
                              np.asarray(W_out, np.float32))
        builder = _build_program_pair
        # prescaled pair C'': tile r = [b*c_{2r+1} ; (b/a)*c_{2r}]
        bf = b.astype(np.float32)[None, :]
        baf = (b / a).astype(np.float32)[None, :]
    else:
        builder = _build_program

    if "nc" not in _cached:
        _cached["nc"] = builder()
    nc = _cached["nc"]

    in_maps = []
    for i in range(N_CORES):
        Cc = C[i * B:(i + 1) * B]                        # [B, S, H]
        if SCHEME == "pair":
            odd = (Cc[:, 1::2, :] * bf).transpose(1, 2, 0)   # [NPAIR, H, B]
            even = (Cc[:, 0::2, :] * baf).transpose(1, 2, 0)
            C_core = np.ascontiguousarray(
                np.concatenate([odd, even], axis=1))     # [NPAIR, 2H, B]
        else:
            C_core = np.ascontiguousarray(Cc.transpose(1, 2, 0))  # [S, H, B]
        in_maps.append({"in_C": C_core, **wmaps})

    core_ids = list(range(N_CORES))
    _cached["in_maps"] = in_maps
    res = run_bass_kernel_spmd(nc, in_maps, core_ids)

    out = np.empty((B_FULL, S, 1), dtype=np.float32)
    for i in range(N_CORES):
        if SCHEME == "pair":
            dev = res.results[i]["out"].reshape(NSEGP, 2, SEGP, B)
            dev = dev.transpose(0, 2, 1, 3).reshape(S, B)   # [o, b]
        else:
            dev = res.results[i]["out"].reshape(S, B)        # [s, b_local]
        out[i * B:(i + 1) * B, :, 0] = dev.T + b_out[0]
    return out


def _in_maps_for_test(C, wmaps):
    maps = []
    for i in range(N_CORES):
        C_core = np.ascontiguousarray(C[i * B:(i + 1) * B].transpose(1, 2, 0))
        maps.append({"in_C": C_core, **wmaps})
    return maps
